# revision 33
# baseline (speedup 1.0000x reference)
"""MoE router kernel for Trainium2 (8 NeuronCores, SPMD data-parallel).

Computes, for x [B,S,H] and gate_w [E,H]:
    logits = x @ gate_w.T           # [B,S,E]
    p = softmax(logits, -1)
    w, i = top_k(p, 2); w = w / w.sum(-1, keepdims=True)

Math used on-device: renormalized top-2 softmax weights collapse to
    w1 = sigmoid(l1 - l2), w2 = 1 - w1
where l1 >= l2 are the top-2 logits, so the full softmax is never needed.

Sharding: tokens (B*S = 16384) split evenly across 8 cores; gate weights
replicated. Per core: 2048 tokens x 4096 hidden.

v5 design (weight-stationary split-precision bf16, token-group pipeline):
  The host splits x and gate_w into bf16 hi/lo pairs (16 mantissa bits
  total; fp32-grade logits, sigma ~ 4e-6, verified zero top-2 flips on
  the problem distribution) and pre-transposes each core's slice into
  [group, quad, 128, chunk, {hi,lo}, tok] layout with 16 KB contiguous
  per-partition DMA lines.

  Tokens are processed in 4 sequential groups of 512. Per group, the PE
  accumulates logitsT [64, 512] in one PSUM bank over 32 contraction
  chunks (3 bf16 matmuls each: wh*xh + wh*xl + wl*xh, 1 cycle/row).
  Each group's tail (drain, PE back-transpose, DVE max8/max_index,
  sigmoid, output DMA) is emitted inside the NEXT group's stream so it
  hides under DMA/PE; only the last group's tail is exposed.
"""

import sys

sys.path.insert(0, "/opt/trn_rl_repo")

import numpy as np
import ml_dtypes

import concourse.bass as bass
import concourse.mybir as mybir
import concourse.tile as tile
from concourse.bass_utils import run_bass_kernel_spmd
import orjson
import concourse.bass_utils as _bu
import concourse.bass2jax as _b2j

_orig_compile_bir = _bu.compile_bir_kernel


def _legalize_waits(bir_json: bytes) -> bytes:
    """This walrus build allows only ONE sync-wait per compute
    instruction; move excess waits onto a Drain inserted just before
    (Drain accepts many waits)."""
    m = orjson.loads(bir_json)
    changed = False
    for fn in m["functions"]:
        for blk in fn["blocks"]:
            out = []
            for inst in blk["instructions"]:
                si = inst.get("sync_info")
                w = (si or {}).get("on_wait") or []
                if len(w) > 1:
                    for k, wk in enumerate(w[:-1]):
                        out.append({
                            "debug": inst.get("debug", 0),
                            "engine": inst["engine"],
                            "ins": [], "outs": [],
                            "name": inst["name"] + f"-lw{k}",
                            "opcode": "Drain",
                            "sync_info": {"on_update": [], "on_wait": [wk]},
                        })
                    si["on_wait"] = w[-1:]
                    changed = True
                out.append(inst)
            blk["instructions"] = out
    return orjson.dumps(m) if changed else bir_json


def _compile_bir_legalized(bir_json, tmpdir, neff_name="file.neff"):
    return _orig_compile_bir(_legalize_waits(bir_json), tmpdir, neff_name)


_bu.compile_bir_kernel = _compile_bir_legalized
_b2j.compile_bir_kernel = _compile_bir_legalized

F32 = mybir.dt.float32
BF16 = mybir.dt.bfloat16
U32 = mybir.dt.uint32
Alu = mybir.AluOpType
BF = ml_dtypes.bfloat16

B, S, H, E = 4, 4096, 4096, 64
N_CORES = 8
P = 128                      # partitions / tile height
TOK_TOTAL = B * S            # 16384
TOK = TOK_TOTAL // N_CORES   # 2048 tokens per core
NCH = H // P                 # 32 contraction chunks of 128
NG = 4                       # token groups
TG = TOK // NG               # 512 tokens per group (= 1 PSUM bank)
QC = 4                       # DMA quads per group
CCQ = NCH // QC              # 8 chunks per quad
NT = TOK // P                # 16 output tiles of 128 tokens
TPG = TG // P                # 4 tiles per group


def build_nc(tok: int = TOK):
    """Build the per-core Bass program (SPMD: same program, 8 cores)."""
    nc = bass.Bass()

    xt_ext = nc.declare_dram_parameter("xt", [NG, QC, P, CCQ, 2, TG], BF16,
                                       isOutput=False)
    wth_ext = nc.declare_dram_parameter("wth", [P, NCH, E], BF16,
                                        isOutput=False)
    wtl_ext = nc.declare_dram_parameter("wtl", [P, NCH, E], BF16,
                                        isOutput=False)
    id_ext = nc.declare_dram_parameter("ident", [P, P], F32, isOutput=False)
    ow_ext = nc.declare_dram_parameter("out_w", [P, NT, 2], F32,
                                       isOutput=True)
    oi_ext = nc.declare_dram_parameter("out_i", [P, NT, 8], U32,
                                       isOutput=True)

    with tile.TileContext(nc) as tc:
        with (
            tc.tile_pool(name="consts", bufs=1) as consts,
            tc.tile_pool(name="xin", bufs=6) as xin,
            tc.tile_pool(name="psl", bufs=1, space="PSUM") as psl,
            tc.tile_pool(name="pst", bufs=2, space="PSUM") as pst,
            tc.tile_pool(name="small", bufs=4) as small,
            tc.tile_pool(name="outp", bufs=1) as outp,
        ):
            # Sync-queue order is the ring order: first x quad, then the
            # wh/wl halves of the gate weight. The first matmuls need only
            # quad0 + wh; wl is first consumed ~16 matmuls later.
            x_sb0 = xin.tile([P, CCQ, 2, TG], BF16, name="x_sb")
            nc.sync.dma_start(x_sb0[:], xt_ext[0, 0])
            wth_sb = consts.tile([P, NCH, E], BF16)
            nc.sync.dma_start(wth_sb[:], wth_ext[:])
            wtl_sb = consts.tile([P, NCH, E], BF16)
            nc.sync.dma_start(wtl_sb[:], wtl_ext[:])
            id_sb = consts.tile([P, P], F32)
            nc.scalar.dma_start(id_sb[:], id_ext[:])

            # Primers: walrus allows only ONE sync-wait per compute
            # instruction. Give every engine a first op with no other
            # dependency (const APs are pre-TileContext, untracked), and
            # absorb the wt-DMA sem into a throwaway PE op. The ident-DMA
            # sem rides on the first tail transpose (single wait, arrives
            # long before the tail).
            prim = consts.tile([P, 2], F32)
            nc.vector.memset(prim[:, 0:1], 0.0)
            nc.scalar.copy(prim[:, 1:2], nc.const_aps.tensor(1.0, (P, 1)))
            with tc.tile_pool(name="scr", bufs=1, space="PSUM") as scr_pool:
                scr2 = scr_pool.tile([E, E], BF16)
                nc.tensor.matmul(scr2[:], wth_sb[:, 0, :],
                                 wth_sb[:, 0, :],
                                 is_transpose=True, start=True, stop=True)

            # logitsT accumulators: one [64, TG] bank per token group.
            lgT = [psl.tile([E, TG], F32, name=f"lgT{g}") for g in range(NG)]
            mxa = outp.tile([P, NT, 8], F32)
            ixa = outp.tile([P, NT, 8], U32)
            d_all = outp.tile([P, NT], F32)
            owa = outp.tile([P, NT, 2], F32)

            def emit_quad(g, qc):
                if g == 0 and qc == 0:
                    x_sb = x_sb0          # DMA'd ahead of the weights
                else:
                    x_sb = xin.tile([P, CCQ, 2, TG], BF16, name="x_sb")
                    nc.sync.dma_start(x_sb[:], xt_ext[g, qc])
                # All wh passes first, wl passes second: the wl weight half
                # arrives after wh and is only needed ~16 matmuls in.
                for cc in range(CCQ):
                    c = CCQ * qc + cc
                    nc.tensor.matmul(lgT[g][:], wth_sb[:, c, :],
                                     x_sb[:, cc, 0, :],
                                     start=(c == 0), stop=False)
                    nc.tensor.matmul(lgT[g][:], wth_sb[:, c, :],
                                     x_sb[:, cc, 1, :],
                                     start=False, stop=False)
                for cc in range(CCQ):
                    c = CCQ * qc + cc
                    nc.tensor.matmul(lgT[g][:], wtl_sb[:, c, :],
                                     x_sb[:, cc, 0, :],
                                     start=False,
                                     stop=(c == NCH - 1))

            def emit_tail(g):
                # Drain on DVE: the scalar queue carries out-DMAs that wait
                # on the sigmoid and would delay this drain (and with it the
                # PE back-transposes queued behind the stream matmuls).
                lgT_sb = small.tile([E, TG], F32, name="lgT_sb")
                nc.vector.tensor_copy(lgT_sb[:], lgT[g][:])
                for j in range(TPG):
                    t = g * TPG + j
                    lg_ps = pst.tile([P, E], F32, name="lg_ps")
                    nc.tensor.matmul(lg_ps[:], lgT_sb[:, j * P:(j + 1) * P],
                                     id_sb[0:E, 0:E], is_transpose=True,
                                     start=True, stop=True)
                    nc.vector.max(mxa[:, t, :], lg_ps[:])
                    nc.vector.max_index(ixa[:, t, :], mxa[:, t, :], lg_ps[:])
                sl = slice(g * TPG, (g + 1) * TPG)
                nc.vector.scalar_tensor_tensor(
                    d_all[:, sl], mxa[:, sl, 0], 1.0, mxa[:, sl, 1],
                    Alu.mult, Alu.subtract)
                nc.scalar.activation(owa[:, sl, 0], d_all[:, sl],
                                     mybir.ActivationFunctionType.Sigmoid)
                nc.vector.tensor_scalar(owa[:, sl, 1], owa[:, sl, 0],
                                        -1.0, 1.0, Alu.mult, Alu.add)
                # Keep the Sync queue exclusively for input-stream DMAs: an
                # output DMA here would sit in the Sync FIFO waiting on the
                # sigmoid and stall every later input-quad issue behind it.
                nc.scalar.dma_start(ow_ext[:, sl, :], owa[:, sl, :])
                nc.scalar.dma_start(oi_ext[:, sl, :], ixa[:, sl, :])

            for g in range(NG):
                for qc in range(QC):
                    emit_quad(g, qc)
                    # Previous group's tail hides under this stream.
                    if qc == 1 and g >= 1:
                        emit_tail(g - 1)
            emit_tail(NG - 1)

    return nc


_NC_CACHE = {}


def _get_nc(tok: int):
    if tok not in _NC_CACHE:
        _NC_CACHE[tok] = build_nc(tok)
    return _NC_CACHE[tok]


def make_in_maps(x: np.ndarray, gate_w: np.ndarray):
    """Shard + split inputs into per-core input maps."""
    xf = np.ascontiguousarray(x.reshape(TOK_TOTAL, H), dtype=np.float32)
    xh = xf.astype(BF)
    xl = (xf - xh.astype(np.float32)).astype(BF)

    w32 = np.asarray(gate_w, np.float32)
    wh = w32.astype(BF)
    wl = (w32 - wh.astype(np.float32)).astype(BF)
    # wt*[p, c, e] = w_*[e, 128*c + p]
    wth = np.ascontiguousarray(wh.T.reshape(NCH, P, E).transpose(1, 0, 2))
    wtl = np.ascontiguousarray(wl.T.reshape(NCH, P, E).transpose(1, 0, 2))

    ident = np.eye(P, dtype=np.float32)
    maps = []
    for i in range(N_CORES):
        sl = slice(i * TOK, (i + 1) * TOK)
        # xt[g, qc, p, cc, s, t] = x_s[token TG*g + t, 128*(CCQ*qc+cc) + p]
        # x_s[sl].T is [H, TOK]; reshape H -> (QC, CCQ, P), TOK -> (NG, TG)
        xhT = np.ascontiguousarray(xh[sl].T).reshape(QC, CCQ, P, NG, TG)
        xlT = np.ascontiguousarray(xl[sl].T).reshape(QC, CCQ, P, NG, TG)
        xt = np.empty((NG, QC, P, CCQ, 2, TG), dtype=BF)
        xt[:, :, :, :, 0, :] = xhT.transpose(3, 0, 2, 1, 4)
        xt[:, :, :, :, 1, :] = xlT.transpose(3, 0, 2, 1, 4)
        maps.append({"xt": xt, "wth": wth, "wtl": wtl, "ident": ident})
    return maps


def kernel(x, gate_w, _trace: bool = False):
    x = np.asarray(x, dtype=np.float32)
    gate_w = np.asarray(gate_w, dtype=np.float32)
    nc = _get_nc(TOK)
    in_maps = make_in_maps(x, gate_w)
    res = run_bass_kernel_spmd(
        nc, in_maps, core_ids=list(range(N_CORES)), trace=_trace
    )
    # Device returns [128, NT, k] partition-major; unpermute to [tok, 2].
    out_w = np.concatenate([
        res.results[i]["out_w"].transpose(1, 0, 2).reshape(TOK, 2)
        for i in range(N_CORES)
    ])
    out_i = np.concatenate([
        res.results[i]["out_i"][:, :, 0:2].transpose(1, 0, 2).reshape(TOK, 2)
        for i in range(N_CORES)
    ])
    topk_weights = out_w.reshape(B, S, 2)
    topk_indices = out_i.astype(np.int32).reshape(B, S, 2)
    if _trace:
        kernel._last_result = res
    return topk_weights, topk_indices
